# revision 9
# baseline (speedup 1.0000x reference)
"""ConvAttention (XCA-style channel attention) Trainium2 Bass kernel.

Reference computation (per batch element n, DIM=192, HEADS=6, H=W=128):
    qkv = conv3x3(x, qkv_w)                  # [576, H, W]
    q, k, v = split(qkv)                     # each [192, H*W]
    q = q / max(||q||_hw, eps); k likewise   # L2 norm over spatial
    attn = softmax(q @ k^T * temp, axis=-1)  # per-head [32, 32]
    out = attn @ v                           # [192, H*W]
    out = proj_w @ out + proj_b

Strategy (8 cores, data-parallel over batch N=8 -> 1 image per core):
  - 3x3 conv done as shifted matmuls over a zero-padded flat bf16 image.
    The contraction axis (192 ch x 9 taps = 1728) is re-chunked into 14
    dense K=128 chunks: 9 chunks = channels 0-127 of each tap; 5 chunks
    pack channels 128-191 of two adjacent taps. The host uploads the
    image as [384, XF]: rows 0-127 = ch 0-127; rows 128-255 =
    [ch 128-191 | same shifted +1]; rows 256-383 = [same | shifted +128]
    so each SBUF window is one DMA per source.
  - Phase A computes q,k SPATIAL-major ([128 positions, 384 ch] psum
    tiles); the Gram J^T J (J=[q|k]) accumulates directly on PE with the
    spatial dim as contraction. Norms = Gram diagonal.
  - Phase B builds the 6 32x32 softmax blocks A from the Gram on
    DVE/ACT, then folds attention AND projection into effective conv
    weights: FW_t = proj_w @ BlockDiag(A) @ Wv_t  (on PE, tiny).
  - Phase C: out = conv3x3(x, FW) + bias, also SPATIAL-major (M=128
    positions, N=192 outputs — no M under-utilization); the output is
    written [16384, 192] and transposed on the host. v is never
    materialized.
"""

import numpy as np
import ml_dtypes

import concourse.bass as bass
import concourse.tile as tile
from concourse import bacc, mybir
from concourse.bass_utils import run_bass_kernel_spmd

F32 = mybir.dt.float32
BF16 = mybir.dt.bfloat16
AF = mybir.ActivationFunctionType
ALU = mybir.AluOpType
AX = mybir.AxisListType

DIM = 192
HEADS = 6
H = W = 128
HP = WP = 130  # padded
XF = 17028  # flat padded length + 128 slop for the +128-shifted window
NPOS = H * W
NCORES = 8
EPS = 1e-12

# contraction chunks: (source, tap) where source 0=lo, 1=hi shifted +1,
# 2=hi shifted +128. Chunk 13 pairs hi@t8 with zero weights.
CHUNKS = [(0, t) for t in range(9)] + [(1, 0), (2, 2), (1, 4), (1, 6), (1, 8)]

_CACHE = {}


def _emit(tc):
    nc = tc.nc
    xp = nc.dram_tensor("xp", [384, XF], BF16, kind="ExternalInput").ap()
    wqk = nc.dram_tensor("wqk", [128, 14, 384], BF16, kind="ExternalInput").ap()
    wvs = nc.dram_tensor("wvs", [DIM, 14, 128], BF16, kind="ExternalInput").ap()
    wpt = nc.dram_tensor("wpt", [DIM, 192], BF16, kind="ExternalInput").ap()
    smalls = nc.dram_tensor("smalls", [DIM, 194], F32, kind="ExternalInput").ap()
    bcast = nc.dram_tensor("bcast", [128, 320], F32, kind="ExternalInput").ap()
    out = nc.dram_tensor("out", [NPOS, DIM], F32, kind="ExternalOutput").ap()
    tkd = nc.dram_tensor("tkd", [1, 192], F32).ap()  # internal bounce

    import contextlib

    def _load_windows(pool, g):
        b = 520 * g
        xb = []
        for i, nm in enumerate(("xb_lo", "xb_h1", "xb_h2")):
            t = pool.tile([128, 6, WP], BF16, tag=nm, name=nm)
            nc.sync.dma_start(
                t[:],
                xp[128 * i : 128 * (i + 1), b : b + 780].rearrange(
                    "p (a c) -> p a c", a=6, c=WP))
            xb.append(t)
        return xb

    with contextlib.ExitStack() as ctx:
        # ---------- persistent SBUF ----------
        pers = ctx.enter_context(tc.tile_pool(name="pers", bufs=1))
        wqk_sb = pers.tile([128, 14, 384], BF16)
        wvs_lo = pers.tile([128, 14, 128], BF16)
        wvs_hi = pers.tile([64, 14, 128], BF16)
        wpt_lo = pers.tile([128, 192], BF16)
        wpt_hi = pers.tile([64, 192], BF16)
        smalls_lo = pers.tile([128, 194], F32)
        smalls_hi = pers.tile([64, 194], F32)
        bcast_sb = pers.tile([128, 320], F32)
        fw_sb = pers.tile([128, 14, 192], BF16)

        tempq_lo = smalls_lo[:, 1:2]
        tempq_hi = smalls_hi[:, 1:2]
        mask_lo = smalls_lo[:, 2:194]
        mask_hi = smalls_hi[:, 2:194]
        ident_sb = bcast_sb[:, 0:128]
        biasb_sb = bcast_sb[:, 128:320]

        # wqk first: its transfer dominates what the first matmul waits on.
        nc.sync.dma_start(wqk_sb[:], wqk[:])

        # ---------- Gram accumulator (PSUM, lives through phase A+B) ----------
        gram_pool = ctx.enter_context(
            tc.tile_pool(name="gram", bufs=1, space="PSUM")
        )
        gram0 = gram_pool.tile([128, 384], F32)
        gram1 = gram_pool.tile([128, 256], F32)
        gram2 = gram_pool.tile([128, 128], F32)

        # ================= phase A: q,k conv + gram =================
        with tc.tile_pool(name="xbp", bufs=2) as xbp, \
             tc.tile_pool(name="jp", bufs=3) as jp, \
             tc.tile_pool(name="pqkp", bufs=3, space="PSUM") as pqkp:
            for g in range(32):  # 4 output rows per group
                xsrc = _load_windows(xbp, g)
                for r in range(4):
                    y = 4 * g + r
                    pqk = pqkp.tile([128, 384], F32, tag="pqk")
                    for j, (srci, t) in enumerate(CHUNKS):
                        dy, dx = divmod(t, 3)
                        nc.tensor.matmul(
                            pqk[:], lhsT=xsrc[srci][:, r + dy, dx : dx + 128],
                            rhs=wqk_sb[:, j, :], start=(j == 0), stop=(j == 13))
                    jt = jp.tile([128, 384], BF16, tag="jt")
                    nc.vector.tensor_copy(jt[:], pqk[:])
                    st, sp = (y == 0), (y == 127)
                    nc.tensor.matmul(gram0[:], lhsT=jt[:, 0:128], rhs=jt[:],
                                     start=st, stop=sp)
                    nc.tensor.matmul(gram1[:], lhsT=jt[:, 128:256],
                                     rhs=jt[:, 128:384], start=st, stop=sp)
                    nc.tensor.matmul(gram2[:], lhsT=jt[:, 256:384],
                                     rhs=jt[:, 256:384], start=st, stop=sp)

        # remaining persistent loads (needed from phase B on)
        nc.sync.dma_start(wvs_lo[:], wvs[0:128])
        nc.sync.dma_start(wvs_hi[:], wvs[128:192])
        nc.sync.dma_start(wpt_lo[:], wpt[0:128])
        nc.sync.dma_start(wpt_hi[:], wpt[128:192])
        nc.sync.dma_start(smalls_lo[:], smalls[0:128])
        nc.sync.dma_start(smalls_hi[:], smalls[128:192])
        nc.sync.dma_start(bcast_sb[:], bcast[:])

        # ========== phase B: norms, softmax, fold attn+proj into conv =======
        with tc.tile_pool(name="phb", bufs=1) as phb, \
             tc.tile_pool(name="pmtp", bufs=1, space="PSUM") as pmtp:
            grams = (gram0, gram1, gram2)
            sqall = phb.tile([128, 3], F32)
            for m in range(3):
                dtmp = phb.tile([128, 128], F32, tag=f"dtmp{m}",
                                name=f"dtmp{m}")
                nc.vector.tensor_mul(dtmp[:], grams[m][:, 0:128], ident_sb)
                nc.vector.tensor_reduce(sqall[:, m : m + 1], dtmp[:],
                                        axis=AX.X, op=ALU.add)
            nrm = phb.tile([128, 3], F32)
            nc.scalar.sqrt(nrm[:], sqall[:])
            nc.vector.tensor_scalar_max(nrm[:], nrm[:], EPS)
            inv_all = phb.tile([128, 3], F32)
            nc.vector.reciprocal(inv_all[:], nrm[:])

            # k-channel inverse norms -> one [1,192] row via DRAM bounce,
            # then broadcast to 128 partitions.
            nc.sync.dma_start(tkd[0:1, 0:64], inv_all[64:128, 1:2])
            nc.sync.dma_start(tkd[0:1, 64:192], inv_all[:, 2:3])
            colfac = phb.tile([128, 192], F32)
            nc.sync.dma_start(colfac[:], tkd[0:1, :].to_broadcast((128, 192)))

            rowA = phb.tile([128, 1], F32)
            nc.vector.tensor_mul(rowA[:], inv_all[:, 0:1], tempq_lo)
            rowB = phb.tile([64, 1], F32)
            nc.vector.tensor_mul(rowB[:], inv_all[0:64, 1:2], tempq_hi)

            a_chunks = []
            for name, rows, gsl, rowfac, msk in (
                ("A0", 128, gram0[:, 192:384], rowA, mask_lo),
                ("A1", 64, gram1[0:64, 64:256], rowB, mask_hi),
            ):
                tl = phb.tile([rows, 192], F32, tag=f"tl{name}",
                              name=f"tl{name}")
                nc.vector.tensor_mul(tl[:], gsl, colfac[0:rows, :])
                ex = phb.tile([rows, 192], F32, tag=f"ex{name}",
                              name=f"ex{name}")
                nc.scalar.activation(ex[:], tl[:], AF.Exp, scale=rowfac[:])
                nc.vector.tensor_mul(ex[:], ex[:], msk)
                den = phb.tile([rows, 1], F32, tag=f"den{name}",
                               name=f"den{name}")
                nc.vector.tensor_reduce(den[:], ex[:], axis=AX.X, op=ALU.add)
                rden = phb.tile([rows, 1], F32, tag=f"rden{name}",
                                name=f"rden{name}")
                nc.vector.reciprocal(rden[:], den[:])
                ab = phb.tile([rows, 192], BF16, tag=f"ab{name}",
                              name=f"ab{name}")
                nc.vector.tensor_scalar_mul(ab[:], ex[:], rden[:])
                a_chunks.append(ab)
            a0, a1 = a_chunks

            # M^T = BlockDiag(A)^T @ proj_w^T   -> [192(d), 192(o)]
            pmt0 = pmtp.tile([128, 192], F32, tag="pmt0")
            pmt1 = pmtp.tile([64, 192], F32, tag="pmt1")
            nc.tensor.matmul(pmt0[:], lhsT=a0[:, 0:128], rhs=wpt_lo[:],
                             start=True, stop=False)
            nc.tensor.matmul(pmt0[:], lhsT=a1[:, 0:128], rhs=wpt_hi[:],
                             start=False, stop=True)
            nc.tensor.matmul(pmt1[:], lhsT=a0[:, 128:192], rhs=wpt_lo[:],
                             start=True, stop=False)
            nc.tensor.matmul(pmt1[:], lhsT=a1[:, 128:192], rhs=wpt_hi[:],
                             start=False, stop=True)
            mt0 = phb.tile([128, 192], BF16)
            mt1 = phb.tile([64, 192], BF16)
            nc.vector.tensor_copy(mt0[:], pmt0[:])
            nc.vector.tensor_copy(mt1[:], pmt1[:])

            # FW^T[(c,t), o] = sum_d Wv_stack[d,(c,t)] * M^T[d, o]
            for j in range(14):
                pfw = pmtp.tile([128, 192], F32, tag="pfw", name="pfw", bufs=2)
                nc.tensor.matmul(pfw[:], lhsT=wvs_lo[:, j, :], rhs=mt0[:],
                                 start=True, stop=False)
                nc.tensor.matmul(pfw[:], lhsT=wvs_hi[:, j, :], rhs=mt1[:],
                                 start=False, stop=True)
                nc.vector.tensor_copy(fw_sb[:, j, :], pfw[:])

        # ===== phase C: out[pos, o] = conv3x3(x, FW) + bias (spatial-major) ==
        with tc.tile_pool(name="xcp", bufs=2) as xcp, \
             tc.tile_pool(name="obp", bufs=2) as obp, \
             tc.tile_pool(name="pcp", bufs=3, space="PSUM") as pcp:
            for g in range(32):
                xsrc = _load_windows(xcp, g)
                obg = obp.tile([128, 4, 192], F32, tag="obg", name="obg")
                for r in range(4):
                    pc = pcp.tile([128, 192], F32, tag="pc", name="pc")
                    for j, (srci, t) in enumerate(CHUNKS):
                        dy, dx = divmod(t, 3)
                        nc.tensor.matmul(
                            pc[:], lhsT=xsrc[srci][:, r + dy, dx : dx + 128],
                            rhs=fw_sb[:, j, :], start=(j == 0), stop=(j == 13))
                    nc.vector.tensor_add(obg[:, r, :], pc[:], biasb_sb)
                nc.sync.dma_start(
                    out[512 * g : 512 * (g + 1), :].rearrange(
                        "(a p) o -> p a o", a=4, p=128),
                    obg[:])


def build_program():
    if "nc" in _CACHE:
        return _CACHE["nc"]
    nc = bacc.Bacc("TRN2", target_bir_lowering=False, debug=False,
                   num_devices=NCORES)
    with tile.TileContext(nc) as tc:
        _emit(tc)
    nc.compile()
    _CACHE["nc"] = nc
    return nc


def _pack_qk_weights(w):
    """w: [384, 192, 3, 3] fp32 -> [128, 14, 384] bf16 chunk layout."""
    nout = w.shape[0]
    wt = np.ascontiguousarray(w.transpose(1, 2, 3, 0).reshape(DIM, 9, nout))
    packed = np.zeros((128, 14, nout), dtype=np.float32)
    for j in range(9):
        packed[:, j, :] = wt[0:128, j, :]
    for j, t in enumerate((0, 2, 4, 6, 8)):
        packed[0:64, 9 + j, :] = wt[128:192, t, :]
        if t + 1 < 9:
            packed[64:128, 9 + j, :] = wt[128:192, t + 1, :]
    return np.ascontiguousarray(packed).astype(ml_dtypes.bfloat16)


def _pack_v_stack(w):
    """w: [192(d), 192(c), 3, 3] fp32 -> [192(d), 14, 128] bf16: Wv arranged
    by contraction-chunk rows so FW^T chunks come out of one matmul."""
    # wt[d, t, c]
    wt = np.ascontiguousarray(w.transpose(0, 2, 3, 1).reshape(DIM, 9, DIM))
    packed = np.zeros((DIM, 14, 128), dtype=np.float32)
    for j in range(9):
        packed[:, j, :] = wt[:, j, 0:128]
    for j, t in enumerate((0, 2, 4, 6, 8)):
        packed[:, 9 + j, 0:64] = wt[:, t, 128:192]
        if t + 1 < 9:
            packed[:, 9 + j, 64:128] = wt[:, t + 1, 128:192]
    return np.ascontiguousarray(packed).astype(ml_dtypes.bfloat16)


def prep_in_maps(x, qkv_w, proj_w, proj_b, temperature):
    bf16 = ml_dtypes.bfloat16
    n = x.shape[0]
    assert x.shape == (n, DIM, H, W)
    # padded flat image + shifted copies of the high channels:
    # rows 0-127 ch 0-127; 128-191 ch 128-191; 192-255 same +1;
    # 256-319 same; 320-383 same +128.
    xpad = np.zeros((n, 384, XF), dtype=bf16)
    tmp = np.zeros((n, DIM, HP, WP), dtype=bf16)
    tmp[:, :, 1 : H + 1, 1 : W + 1] = x.astype(bf16)
    flat = tmp.reshape(n, DIM, HP * WP)
    xpad[:, 0:192, : HP * WP] = flat
    xpad[:, 192:256, : HP * WP - 1] = flat[:, 128:192, 1:]
    xpad[:, 256:320, : HP * WP] = flat[:, 128:192, :]
    xpad[:, 320:384, : HP * WP - 128] = flat[:, 128:192, 128:]
    wqk = _pack_qk_weights(qkv_w[: 2 * DIM])
    wvs = _pack_v_stack(qkv_w[2 * DIM :])
    wpt = np.ascontiguousarray(proj_w[:, :, 0, 0].T).astype(bf16)
    tq = np.repeat(np.asarray(temperature, np.float32).reshape(HEADS),
                   DIM // HEADS)
    mask = np.kron(np.eye(HEADS, dtype=np.float32),
                   np.ones((DIM // HEADS, DIM // HEADS), np.float32))
    smalls = np.zeros((DIM, 194), dtype=np.float32)
    smalls[:, 0] = proj_b.astype(np.float32)
    smalls[:, 1] = tq
    smalls[:, 2:194] = mask
    bcast = np.zeros((128, 320), dtype=np.float32)
    bcast[:, 0:128] = np.eye(128, dtype=np.float32)
    bcast[:, 128:320] = np.broadcast_to(proj_b.astype(np.float32), (128, DIM))
    shared = {"wqk": wqk, "wvs": wvs, "wpt": wpt, "smalls": smalls,
              "bcast": bcast}
    return [{"xp": np.ascontiguousarray(xpad[i]), **shared} for i in range(n)]


def kernel(x, qkv_w, proj_w, proj_b, temperature):
    x = np.asarray(x, np.float32)
    qkv_w = np.asarray(qkv_w, np.float32)
    proj_w = np.asarray(proj_w, np.float32)
    proj_b = np.asarray(proj_b, np.float32)
    temperature = np.asarray(temperature, np.float32)
    nc = build_program()
    in_maps = prep_in_maps(x, qkv_w, proj_w, proj_b, temperature)
    res = run_bass_kernel_spmd(nc, in_maps, core_ids=list(range(NCORES)))
    outs = [
        res.results[i]["out"].reshape(H, W, DIM).transpose(2, 0, 1)
        for i in range(NCORES)
    ]
    return np.stack(outs, axis=0).astype(np.float32)
